# revision 12
# baseline (speedup 1.0000x reference)
"""Trainium2 Bass kernel for nn_ArgumentLogits (ragged argument logits head).

Self-contained: hardcodes all shapes. Strategy: data-parallel over batch
(8 cores x 32 batches). All ragged structure derives from int inputs, so the
host computes the index plumbing (as the reference itself does), packs padded
feature-major layouts per core, and the device graph (identical across cores,
SPMD) does every FLOP: dense st/lng chain, keys matmul, per-batch local-logit
matmuls, embedding-key norm stats, and the def-tile-oriented global matmul
with 1/||gk|| fused into the PSUM eviction as a per-partition scale.
"""

import math
import time

import numpy as np
import ml_dtypes

import concourse.bass as bass
import concourse.mybir as mybir
import concourse.tile as tile
from concourse import bacc
from concourse.bass_utils import run_bass_kernel_spmd
from concourse.masks import make_identity

BS = 256
MAX_ARGS = 8
CTX_DIM = 128
NODE_DIM = 128
HIDDEN = 512
STATE_DIM = 512
TAC_DIM = 128
TOTAL_CTX = 131072
N_CLASS = 30000
DEF_NUM = 20000
CTX_VAL_DIM = 256
DIM = CTX_DIM + 1 + NODE_DIM  # 257
N_CORES = 8
BPC = BS // N_CORES  # batches per core = 32
N_DEF_TILES = (DEF_NUM + 127) // 128  # 157
DEF_PAD = N_DEF_TILES * 128  # 20096

BF16 = mybir.dt.bfloat16
F32 = mybir.dt.float32
NP_BF16 = ml_dtypes.bfloat16

FLOAT_KEYS = ("ctx_vals", "state_emb", "tactic_emb", "emb_table", "W_key",
              "b_key", "W_st", "b_st", "W_q", "b_q")


# ---------------------------------------------------------------- host plumbing

def _build_indices(ctx_ids, arg_cnt):
    """Mirror of the reference's host-side ragged index reconstruction."""
    ctx_ids = np.asarray(ctx_ids)
    arg_cnt = np.asarray(arg_cnt)
    arguments_i = np.repeat(np.arange(BS), arg_cnt)
    total_args = arguments_i.shape[0]
    ctx_lens = np.bincount(ctx_ids, minlength=BS)
    ctx_starts = np.concatenate([[0], np.cumsum(ctx_lens)[:-1]])
    arg_ctx_lens = ctx_lens[arguments_i]
    rows = np.repeat(np.arange(total_args), arg_ctx_lens)
    return arguments_i, total_args, ctx_lens, ctx_starts, arg_ctx_lens, rows


def _plan(ctx_ids, arg_cnt):
    """Choose padded sizes + batch->core assignment (balanced by arg count)."""
    arg_cnt = np.asarray(arg_cnt)
    ctx_lens = np.bincount(np.asarray(ctx_ids), minlength=BS)
    len_pad = max(128, int(math.ceil(ctx_lens.max() / 128.0)) * 128)

    # greedy LPT partition of batches into 8 groups of exactly BPC,
    # balancing total args per core
    order = np.argsort(-arg_cnt, kind="stable")
    core_args = [0] * N_CORES
    core_batches = [[] for _ in range(N_CORES)]
    for b in order:
        cands = [c for c in range(N_CORES) if len(core_batches[c]) < BPC]
        c = min(cands, key=lambda c: (core_args[c], c))
        core_batches[c].append(int(b))
        core_args[c] += int(arg_cnt[b])
    core_batches = [sorted(cb) for cb in core_batches]
    args_pad = max(128, int(math.ceil(max(core_args) / 32.0)) * 32)
    assert args_pad <= 256
    return dict(len_pad=len_pad, core_batches=core_batches, args_pad=args_pad,
                ctx_lens=ctx_lens)


# ---------------------------------------------------------------- device graph

_GRAPH_CACHE = {}


def build_graph(len_pad, args_pad):
    key = (len_pad, args_pad)
    if key in _GRAPH_CACHE:
        return _GRAPH_CACHE[key]

    LP = len_pad
    A = args_pad
    NNODE = BPC * LP                      # per-core padded ctx nodes
    CHUNK = 4 * LP                        # keys pipeline chunk (multiple of 512)
    NSUB = CHUNK // 512                   # 512-wide psum subchunks per chunk
    G = 512 // A                          # def-tiles per psum bank in global phase
    nloc = (LP + 511) // 512              # local N-chunks per batch

    nc = bacc.Bacc("TRN2", target_bir_lowering=False, debug=False)

    # inputs (per-core shards / replicated)
    ctxT = nc.dram_tensor("ctxT", [2, 128, NNODE], BF16, kind="ExternalInput")
    stinT = nc.dram_tensor("stinT", [128, 5, BPC], BF16, kind="ExternalInput")
    wst = nc.dram_tensor("wst", [128, 5, HIDDEN], BF16, kind="ExternalInput")
    wq = nc.dram_tensor("wq", [128, 4, MAX_ARGS * DIM], BF16, kind="ExternalInput")
    wqn = nc.dram_tensor("wqn", [128, 4, MAX_ARGS], BF16, kind="ExternalInput")
    wkey = nc.dram_tensor("wkey", [128, 2, CTX_DIM], BF16, kind="ExternalInput")
    b_keyC = nc.dram_tensor("b_keyC", [128, 1], F32, kind="ExternalInput")
    b_stT = nc.dram_tensor("b_stT", [128, 4], F32, kind="ExternalInput")
    bq_locT = nc.dram_tensor("bq_locT", [128, MAX_ARGS], F32, kind="ExternalInput")
    bq_gloT = nc.dram_tensor("bq_gloT", [128, MAX_ARGS], F32, kind="ExternalInput")
    b_noneC = nc.dram_tensor("b_noneC", [MAX_ARGS, 1], F32, kind="ExternalInput")
    gkT = nc.dram_tensor("gkT", [128, DEF_PAD], BF16, kind="ExternalInput")
    gkdm = nc.dram_tensor("gkdm", [128, N_DEF_TILES, 128], BF16, kind="ExternalInput")
    sel = nc.dram_tensor("sel", [128, 2, A], BF16, kind="ExternalInput")

    # outputs
    out_local = nc.dram_tensor("out_local", [BPC, MAX_ARGS, LP], F32,
                               kind="ExternalOutput")
    out_none = nc.dram_tensor("out_none", [MAX_ARGS, BPC], F32,
                              kind="ExternalOutput")
    out_glob = nc.dram_tensor("out_glob", [N_DEF_TILES, 128, A], F32,
                              kind="ExternalOutput")

    with tile.TileContext(nc) as tc:
        with (
            tc.tile_pool(name="persist", bufs=1) as persist,
            tc.tile_pool(name="stream", bufs=4) as stream,
            tc.tile_pool(name="gstage", bufs=3) as gstage,
            tc.tile_pool(name="lstage", bufs=2) as lstage,
            tc.tile_pool(name="psmm", bufs=4, space="PSUM") as psmm,
            tc.tile_pool(name="pssm", bufs=2, space="PSUM") as pssm,
        ):
            # ---- resident weights / small inputs
            wst_sb = persist.tile([128, 5, HIDDEN], BF16, tag="wst")
            nc.sync.dma_start(wst_sb[:], wst[:])
            wq_sb = persist.tile([128, 4, MAX_ARGS * DIM], BF16, tag="wq")
            nc.sync.dma_start(wq_sb[:], wq[:])
            wqn_sb = persist.tile([128, 4, MAX_ARGS], BF16, tag="wqn")
            nc.sync.dma_start(wqn_sb[:], wqn[:])
            wkey_sb = persist.tile([128, 2, CTX_DIM], BF16, tag="wkey")
            nc.sync.dma_start(wkey_sb[:], wkey[:])
            stin_sb = persist.tile([128, 5, BPC], BF16, tag="stin")
            nc.sync.dma_start(stin_sb[:], stinT[:])
            sel_sb = persist.tile([128, 2, A], BF16, tag="sel")
            nc.sync.dma_start(sel_sb[:], sel[:])
            bkey_sb = persist.tile([128, 1], F32, tag="bkey")
            nc.sync.dma_start(bkey_sb[:], b_keyC[:])
            bst_sb = persist.tile([128, 4], F32, tag="bst")
            nc.sync.dma_start(bst_sb[:], b_stT[:])
            bloc_sb = persist.tile([128, MAX_ARGS], F32, tag="bloc")
            nc.sync.dma_start(bloc_sb[:], bq_locT[:])
            bglo_sb = persist.tile([128, MAX_ARGS], F32, tag="bglo")
            nc.sync.dma_start(bglo_sb[:], bq_gloT[:])
            bnone_sb = persist.tile([MAX_ARGS, 1], F32, tag="bnone")
            nc.sync.dma_start(bnone_sb[:], b_noneC[:])
            ident_sb = persist.tile([128, 128], BF16, tag="ident")
            make_identity(nc, ident_sb[:])

            # ---- phase 1: st = relu(stin @ W_st + b_st), transposed layout
            st_sb = persist.tile([128, 4, BPC], BF16, tag="st")
            for m in range(4):
                ps = pssm.tile([128, BPC], F32, tag="small")
                for k in range(5):
                    nc.tensor.matmul(ps[:], wst_sb[:, k, m * 128:(m + 1) * 128],
                                     stin_sb[:, k, :], start=(k == 0),
                                     stop=(k == 4))
                nc.scalar.activation(st_sb[:, m, :], ps[:],
                                     mybir.ActivationFunctionType.Relu,
                                     bias=bst_sb[:, m:m + 1])

            # ---- phase 2: local/global queries + none logits (transposed)
            # qT gets 24 pad columns so per-batch lhsT can always be M=32
            qT = persist.tile([128, BPC * MAX_ARGS + 24], BF16, tag="qT")
            nc.vector.memset(qT[:, BPC * MAX_ARGS:], 0)
            gqT = persist.tile([128, BPC * MAX_ARGS], BF16, tag="gqT")
            qT_v = qT[:, :BPC * MAX_ARGS].rearrange("p (b j) -> p b j", j=MAX_ARGS)
            gqT_v = gqT[:].rearrange("p (b j) -> p b j", j=MAX_ARGS)
            for j in range(MAX_ARGS):
                c0 = j * DIM
                ps = pssm.tile([128, BPC], F32, tag="small")
                for k in range(4):
                    nc.tensor.matmul(ps[:], wq_sb[:, k, c0:c0 + CTX_DIM],
                                     st_sb[:, k, :], start=(k == 0), stop=(k == 3))
                nc.scalar.activation(qT_v[:, :, j], ps[:],
                                     mybir.ActivationFunctionType.Identity,
                                     bias=bloc_sb[:, j:j + 1])
                ps2 = pssm.tile([128, BPC], F32, tag="small")
                for k in range(4):
                    nc.tensor.matmul(ps2[:], wq_sb[:, k, c0 + CTX_DIM + 1:c0 + DIM],
                                     st_sb[:, k, :], start=(k == 0), stop=(k == 3))
                nc.scalar.activation(gqT_v[:, :, j], ps2[:],
                                     mybir.ActivationFunctionType.Identity,
                                     bias=bglo_sb[:, j:j + 1])
            psn = pssm.tile([MAX_ARGS, BPC], F32, tag="small")
            for k in range(4):
                nc.tensor.matmul(psn[:], wqn_sb[:, k, :], st_sb[:, k, :],
                                 start=(k == 0), stop=(k == 3))
            none_sb = persist.tile([MAX_ARGS, BPC], F32, tag="none")
            nc.scalar.activation(none_sb[:], psn[:],
                                 mybir.ActivationFunctionType.Identity,
                                 bias=bnone_sb[:, 0:1])
            nc.sync.dma_start(out_none[:], none_sb[:])

            # ---- phase 3: compact global queries via one-hot matmul
            gq_all = persist.tile([128, 2, 128], BF16, tag="gq_all")
            for h in range(2):
                pst = pssm.tile([128, 128], BF16, tag="trans")
                nc.tensor.transpose(pst[:], gqT[:, h * 128:(h + 1) * 128],
                                    ident_sb[:])
                nc.vector.tensor_copy(gq_all[:, h, :], pst[:])
            ps_sel = psmm.tile([128, A], F32, tag="mm")
            for h in range(2):
                nc.tensor.matmul(ps_sel[:], gq_all[:, h, :], sel_sb[:, h, :],
                                 start=(h == 0), stop=(h == 1))
            gq_selT = persist.tile([128, A], BF16, tag="gq_selT")
            nc.vector.tensor_copy(gq_selT[:], ps_sel[:])

            # ---- phase 4: embedding-key norm stats (def-major)
            sumsq = persist.tile([128, N_DEF_TILES], F32, tag="sumsq")
            STATS_CHUNK = 16
            for t0 in range(0, N_DEF_TILES, STATS_CHUNK):
                g = min(STATS_CHUNK, N_DEF_TILES - t0)
                gd = stream.tile([128, STATS_CHUNK, 128], BF16, tag="gkdm")
                nc.sync.dma_start(gd[:, :g, :], gkdm[:, t0:t0 + g, :])
                sq = stream.tile([128, STATS_CHUNK, 128], BF16, tag="sq")
                nc.vector.tensor_mul(sq[:, :g, :], gd[:, :g, :], gd[:, :g, :])
                nc.vector.reduce_sum(sumsq[:, t0:t0 + g], sq[:, :g, :],
                                     axis=mybir.AxisListType.X)
            inv = persist.tile([128, N_DEF_TILES], F32, tag="inv")
            nc.scalar.activation(inv[:], sumsq[:],
                                 mybir.ActivationFunctionType.Sqrt)
            nc.vector.tensor_scalar_add(inv[:], inv[:], 1e-7)
            nc.vector.reciprocal(inv[:], inv[:])

            # ---- phase 5: keys = ctx @ W_key + b_key (feature-major)
            gkT_sb = persist.tile([128, DEF_PAD], BF16, tag="gkT")
            nc.sync.dma_start(gkT_sb[:], gkT[:])
            keysT = persist.tile([128, NNODE], BF16, tag="keysT")
            for c0 in range(0, NNODE, CHUNK):
                cx0 = stream.tile([128, CHUNK], BF16, tag="cx0")
                cx1 = stream.tile([128, CHUNK], BF16, tag="cx1")
                nc.sync.dma_start(cx0[:], ctxT[0, :, c0:c0 + CHUNK])
                nc.sync.dma_start(cx1[:], ctxT[1, :, c0:c0 + CHUNK])
                for s in range(NSUB):
                    ps = psmm.tile([128, 512], F32, tag="mm")
                    sl = slice(s * 512, (s + 1) * 512)
                    nc.tensor.matmul(ps[:], wkey_sb[:, 0, :], cx0[:, sl],
                                     start=True, stop=False)
                    nc.tensor.matmul(ps[:], wkey_sb[:, 1, :], cx1[:, sl],
                                     start=False, stop=True)
                    nc.scalar.activation(keysT[:, c0 + s * 512:c0 + (s + 1) * 512],
                                         ps[:],
                                         mybir.ActivationFunctionType.Identity,
                                         bias=bkey_sb[:, 0:1])

            # ---- phase 6: local logits; 4 batches share one PSUM bank via
            # TensorE col-tiling (batch g -> output partitions 32g..32g+8)
            for b0 in range(0, BPC, 4):
                stg = lstage.tile([128, LP], F32, tag="lst")
                for s in range(nloc):
                    n0 = s * 512
                    n1 = min(LP, n0 + 512)
                    psl = psmm.tile([128, 512], F32, tag="mm")
                    for g in range(4):
                        b = b0 + g
                        nc.tensor.matmul(
                            psl[32 * g:32 * g + 32, :n1 - n0],
                            qT[:, b * MAX_ARGS:b * MAX_ARGS + 32],
                            keysT[:, b * LP + n0:b * LP + n1],
                            start=True, stop=True, tile_position=(0, 32 * g))
                    nc.vector.tensor_copy(stg[:, n0:n1], psl[:, :n1 - n0])
                for g in range(4):
                    nc.sync.dma_start(out_local[b0 + g],
                                      stg[32 * g:32 * g + MAX_ARGS, :])

            # ---- phase 7: global logits, def-tile major, norm fused in evict
            for t0 in range(0, N_DEF_TILES, G):
                g = min(G, N_DEF_TILES - t0)
                psg = psmm.tile([128, G * A], F32, tag="mm")
                for i in range(g):
                    t = t0 + i
                    nc.tensor.matmul(psg[:, i * A:(i + 1) * A],
                                     gkT_sb[:, t * 128:(t + 1) * 128],
                                     gq_selT[:], start=True, stop=True)
                stg = gstage.tile([128, G, A], F32, tag="gst")
                inv_b = inv[:, t0:t0 + g].to_broadcast((128, g, A))
                psg_v = psg[:].rearrange("p (t a) -> p t a", a=A)
                nc.vector.tensor_tensor(stg[:, :g, :], psg_v[:, :g, :], inv_b,
                                        op=mybir.AluOpType.mult)
                nc.sync.dma_start(
                    out_glob[t0:t0 + g].rearrange("t p a -> p t a"),
                    stg[:, :g, :])

    nc.compile()
    _GRAPH_CACHE[key] = nc
    return nc


# ---------------------------------------------------------------- input packing

def pack_inputs(plan, inputs):
    LP = plan["len_pad"]
    A = plan["args_pad"]
    ctx_lens = plan["ctx_lens"]
    ctx_starts = np.concatenate([[0], np.cumsum(ctx_lens)[:-1]])
    arg_cnt = np.asarray(inputs["arg_cnt"])

    f = {k: np.asarray(inputs[k], np.float32) for k in FLOAT_KEYS}
    gc = np.asarray(inputs["global_context"])

    # shared (replicated) tensors
    W_st, b_st, W_q, b_q = f["W_st"], f["b_st"], f["W_q"], f["b_q"]
    wst_r = np.ascontiguousarray(
        W_st.reshape(5, 128, HIDDEN).transpose(1, 0, 2)).astype(NP_BF16)
    wq_r = np.ascontiguousarray(
        W_q.reshape(4, 128, MAX_ARGS * DIM).transpose(1, 0, 2)).astype(NP_BF16)
    none_cols = [j * DIM + CTX_DIM for j in range(MAX_ARGS)]
    wqn_r = np.ascontiguousarray(
        W_q[:, none_cols].reshape(4, 128, MAX_ARGS).transpose(1, 0, 2)
    ).astype(NP_BF16)
    wkey_r = np.ascontiguousarray(
        f["W_key"].reshape(2, 128, CTX_DIM).transpose(1, 0, 2)).astype(NP_BF16)
    b_keyC = f["b_key"].reshape(128, 1)
    b_stT = np.ascontiguousarray(b_st.reshape(4, 128).T)
    bq_locT = np.stack([b_q[j * DIM:j * DIM + CTX_DIM] for j in range(MAX_ARGS)],
                       axis=1)  # [128, 8]
    bq_gloT = np.stack([b_q[j * DIM + CTX_DIM + 1:(j + 1) * DIM]
                        for j in range(MAX_ARGS)], axis=1)
    b_noneC = b_q[none_cols].reshape(MAX_ARGS, 1)

    gk_raw = f["emb_table"][gc]  # [20000, 128] host gather (data movement)
    gk_pad = np.zeros((DEF_PAD, NODE_DIM), np.float32)
    gk_pad[:DEF_NUM] = gk_raw
    gkT = np.ascontiguousarray(gk_pad.T).astype(NP_BF16)  # [128, 20096]
    gkdm = np.ascontiguousarray(
        gk_pad.reshape(N_DEF_TILES, 128, 128).transpose(1, 0, 2)).astype(NP_BF16)

    ctx_vals = f["ctx_vals"]
    state_emb, tactic_emb = f["state_emb"], f["tactic_emb"]

    in_maps = []
    for c in range(N_CORES):
        bl = plan["core_batches"][c]
        big = np.zeros((BPC * LP, CTX_VAL_DIM), np.float32)
        for i, b in enumerate(bl):
            L = int(ctx_lens[b])
            s0 = int(ctx_starts[b])
            big[i * LP:i * LP + L] = ctx_vals[s0:s0 + L]
        ctxT = np.ascontiguousarray(big.T).reshape(2, 128, BPC * LP).astype(NP_BF16)

        stin = np.concatenate([state_emb[bl], tactic_emb[bl]], axis=1)  # [32,640]
        stinT = np.ascontiguousarray(
            stin.T.reshape(5, 128, BPC).transpose(1, 0, 2)).astype(NP_BF16)

        sel_flat = np.zeros((BPC * MAX_ARGS, A), np.float32)
        a_c = 0
        for i, b in enumerate(bl):
            for j in range(int(arg_cnt[b])):
                sel_flat[i * MAX_ARGS + j, a_c] = 1.0
                a_c += 1
        sel_r = np.ascontiguousarray(
            sel_flat.reshape(2, 128, A).transpose(1, 0, 2)).astype(NP_BF16)

        in_maps.append(dict(
            ctxT=ctxT, stinT=stinT, wst=wst_r, wq=wq_r, wqn=wqn_r,
            wkey=wkey_r, b_keyC=b_keyC, b_stT=b_stT, bq_locT=bq_locT,
            bq_gloT=bq_gloT, b_noneC=b_noneC, gkT=gkT, gkdm=gkdm, sel=sel_r,
        ))
    return in_maps


# ---------------------------------------------------------------- assembly

def assemble(plan, results, ctx_ids, arg_cnt):
    arg_cnt = np.asarray(arg_cnt)
    (arguments_i, total_args, ctx_lens, ctx_starts,
     arg_ctx_lens, rows) = _build_indices(ctx_ids, arg_cnt)

    # arg (b, j) -> (core, b_local, a_c)
    where = {}
    for c in range(N_CORES):
        a_c = 0
        for i, b in enumerate(plan["core_batches"][c]):
            for j in range(int(arg_cnt[b])):
                where[(b, j)] = (c, i, a_c)
                a_c += 1

    loc_parts = []
    none_parts = []
    gcols = np.zeros((total_args,), np.int64)
    gcore = np.zeros((total_args,), np.int64)
    a = 0
    for b in range(BS):
        L = int(ctx_lens[b])
        for j in range(int(arg_cnt[b])):
            c, i, a_c = where[(b, j)]
            loc_parts.append(results[c]["out_local"][i, j, :L])
            none_parts.append(results[c]["out_none"][j, i])
            gcore[a] = c
            gcols[a] = a_c
            a += 1

    local_flat = (np.concatenate(loc_parts) if loc_parts
                  else np.zeros((0,), np.float32))
    none_logits = np.asarray(none_parts, np.float32)

    glob = np.empty((total_args, DEF_NUM), np.float32)
    gl = [results[c]["out_glob"].reshape(DEF_PAD, -1)[:DEF_NUM]
          for c in range(N_CORES)]
    for a in range(total_args):
        glob[a] = gl[gcore[a]][:, gcols[a]]

    values = np.concatenate([local_flat, none_logits, glob.reshape(-1)])
    indices = np.concatenate([
        rows.astype(np.int32),
        np.arange(total_args, dtype=np.int32),
        np.repeat(np.arange(total_args, dtype=np.int32), DEF_NUM)])
    return indices, values.astype(np.float32)


# ---------------------------------------------------------------- entry points

_LAST = {}


def kernel(**inputs):
    ctx_ids = np.asarray(inputs["ctx_ids"])
    arg_cnt = np.asarray(inputs["arg_cnt"])
    plan = _plan(ctx_ids, arg_cnt)
    nc = build_graph(plan["len_pad"], plan["args_pad"])
    in_maps = pack_inputs(plan, inputs)
    res = run_bass_kernel_spmd(nc, in_maps, core_ids=list(range(N_CORES)))
    _LAST.update(nc=nc, in_maps=in_maps, plan=plan)
    return assemble(plan, res.results, ctx_ids, arg_cnt)


def bench_exec_ns(n_iters=64, reps=5):
    """Estimate per-NEFF-execution time by running the kernel n_iters times
    inside one jitted program (bass_exec is effectful => serialized)."""
    import jax
    import jax.numpy as jnp
    from jax.sharding import Mesh, PartitionSpec
    from jax.experimental.shard_map import shard_map
    from concourse.bass2jax import (_bass_exec_p, install_neuronx_cc_hook,
                                    partition_id_tensor)

    nc, in_maps = _LAST["nc"], _LAST["in_maps"]
    install_neuronx_cc_hook()

    part_name = nc.partition_id_tensor.name if nc.partition_id_tensor else None
    in_names, out_names, out_avals, zero_outs = [], [], [], []
    for alloc in nc.m.functions[0].allocations:
        if not isinstance(alloc, mybir.MemoryLocationSet):
            continue
        name = alloc.memorylocations[0].name
        if alloc.kind == "ExternalInput":
            if name != part_name:
                in_names.append(name)
        elif alloc.kind == "ExternalOutput":
            out_names.append(name)
            shape = tuple(alloc.tensor_shape)
            dtype = mybir.dt.np(alloc.dtype)
            out_avals.append(jax.core.ShapedArray(shape, dtype))
            zero_outs.append(np.zeros(shape, dtype))
    n_params = len(in_names)
    all_names = in_names + out_names

    bind_names = all_names + ([part_name] if part_name else [])

    def make_body(n):
        def _body(*args):
            operands = list(args)
            if part_name:
                operands.append(partition_id_tensor())
            outs = None
            for _ in range(n):
                outs = _bass_exec_p.bind(
                    *operands,
                    out_avals=tuple(out_avals),
                    in_names=tuple(bind_names),
                    out_names=tuple(out_names),
                    lowering_input_output_aliases=(),
                    sim_require_finite=True,
                    sim_require_nnan=True,
                    nc=nc,
                )
            return tuple(outs)
        return _body

    devices = jax.devices()[:N_CORES]
    mesh = Mesh(np.asarray(devices), ("core",))
    specs = (PartitionSpec("core"),) * (n_params + len(out_names))
    out_specs = (PartitionSpec("core"),) * len(out_names)

    concat_in = [np.concatenate([np.asarray(in_maps[c][k]) for c in range(N_CORES)],
                                axis=0) for k in in_names]
    concat_zero = [np.zeros((N_CORES * z.shape[0], *z.shape[1:]), z.dtype)
                   for z in zero_outs]

    def timed(n):
        fn = jax.jit(shard_map(make_body(n), mesh=mesh, in_specs=specs,
                               out_specs=out_specs, check_rep=False),
                     keep_unused=True)
        out = fn(*concat_in, *concat_zero)
        jax.block_until_ready(out)  # compile + warm
        best = float("inf")
        for _ in range(reps):
            t0 = time.perf_counter()
            out = fn(*concat_in, *concat_zero)
            jax.block_until_ready(out)
            best = min(best, time.perf_counter() - t0)
        return best

    t1 = timed(1)
    tn = timed(n_iters)
    ns = (tn - t1) / (n_iters - 1) * 1e9
    return ns, t1, tn


if __name__ == "__main__":
    import reference
    inputs = {k: np.asarray(v) for k, v in reference.setup_inputs().items()}
    idx, vals = kernel(**inputs)
    print("kernel ran:", idx.shape, vals.shape)
